# revision 5
# baseline (speedup 1.0000x reference)
"""Trainium2 Bass kernel for nn_Encoder (input-attention LSTM encoder).

Contract: kernel(**inputs) takes the FULL unsharded inputs (numpy) and
returns the FULL output (B, T-1, H) float32.  Internally shards the batch
across 8 NeuronCores (pure data parallel), runs a Bass/Tile kernel per
core, and concatenates the per-core outputs.

Math per timestep t (see reference):
    z     = tanh(pre_x + (h @ W1_h.T)[:,None,:] + (c @ W1_s.T)[:,None,:])
    e     = z @ w_attn2 + b_attn2            # (B, N)
    alpha = softmax_n(e)
    x~    = alpha * x_t
    gates = x~ @ W_ih.T + h @ W_hh.T + b     # (B, 4H)
    LSTM update (i, f, g, o) -> h, c

Engine split of the z/e path (2M elems per step per core, the bottleneck):
  - n-groups g1,g3 ([0:6), [26:46)): DVE tensor_tensor add (pre+u, 2x bf16),
    exact ACT tanh.  Group g2 ([6:26)): the add runs on GpSimd (Pool) to
    offload DVE; its tanh runs last so Pool latency hides.
  - n in [46, 64): ONE fused custom DVE op per group computes
    q = ((clip(pre+u,+-L)^2 - L^2)^2 + Q) * clip(pre+u,+-L); the deg-5 lead
    coeff C is folded into the w_attn2 column those n's use (max |C*q-tanh|
    = 3.3e-2, washes out through softmax+LSTM; validated end to end).
State h~/c~ lives only in bf16 (validated numerically), which kills the
state downcasts; h~ is DMA'd out per step in (t,H,b) bf16 layout and the
host does the final transpose + 0.5 scale + f32 cast.
"""

import os

import numpy as np

B, TM1, N, H = 2048, 128, 64, 128
NCORES = 8
BL = B // NCORES  # 256 batch rows per core
CHUNKS = 2        # BL / 128

NACT = 46                 # n's on the exact ACT-tanh path
NDVE = N - NACT           # n's on the fused DVE polynomial path
GD = NDVE // 2            # two DVE-poly groups
# (offset, size, add-engine); tanh/e processed in order g1, g3, g2
AGROUPS = ((0, 6, "dve"), (6, 20, "pool"), (26, 20, "dve"))
AORDER = (0, 2, 1)

# deg-5 fused-tanh fit: P(a) = POLY_C * ac * ((ac^2 - L^2)^2 + Q), ac=clip(a,+-L)
POLY_L = 2.65
POLY_Q = 37.43353826998534
POLY_C = 0.0103122888175463

_cache = {}


def _register_tanh_op():
    """Register the fused add+clip+deg5-tanh custom DVE op (official
    extension point: concourse.dve_ops.OPS)."""
    from concourse import dve_ops
    from concourse.dve_spec import Spec, Src0, Src1, C0, C1, Zero, minn, maxx, lower
    from concourse.dve_uop import DveOpSpec

    name = "TANH5_FUSED_ADD_ANT"
    for o in dve_ops.OPS:
        if o.name == name:
            return o
    a = Src0 + Src1
    ac = maxx(minn(a, C0), Zero - C0)
    s = ac * ac
    g = s - C0 * C0
    body = (g * g + C1) * ac

    def ref(in0, in1, s0, s1, imm2):
        aa = in0 + in1
        acc = np.clip(aa, -s0, s0)
        ss = acc * acc
        return ((((ss - s0 * s0) ** 2) + s1) * acc).astype(np.float32)

    spec = Spec(body=body, reference=ref)
    row = dve_ops._CUSTOM_DVE_ROW_BASE + len(dve_ops.OPS)
    shas = {}
    for ver in ("v3", "v4"):
        uops = lower(spec, ver=ver)
        shas[ver] = DveOpSpec(name=name, opcode=row, uops=uops, rd1_en=True).sha(ver)
    op = dve_ops.DveOp(name, spec, subdim=False, uops_sha=shas)
    dve_ops.OPS.append(op)
    dve_ops.CUSTOM_DVE_SPECS[name] = spec
    dve_ops._SUB_OPCODE_FOR_NAME[name] = row
    return op


def _build(steps=TM1):
    """Trace + compile the per-core Bass kernel. Returns the Bacc object."""
    from contextlib import ExitStack

    import concourse.bass as bass
    import concourse.tile as tile
    from concourse import bacc, mybir
    from concourse.masks import make_identity

    OP = _register_tanh_op()

    f32 = mybir.dt.float32
    zdt = mybir.dt.bfloat16  # dtype of the z path + weights + state
    AF = mybir.ActivationFunctionType
    AL = mybir.AluOpType

    nc = bacc.Bacc("TRN2", target_bir_lowering=False, debug=False,
                   num_devices=NCORES)

    # X_T: (T, N*BL)  — rhs for the pre-matmul (t on partitions)
    # X_B: (TM1, BL, N) — per-step x_t in (b, n) layout, streamed from DRAM
    xt_ap = nc.dram_tensor("XT", [TM1, N * BL], f32, kind="ExternalInput").ap()
    xb_ap = nc.dram_tensor("XB", [TM1, BL, N], f32, kind="ExternalInput").ap()
    w1ht_ap = nc.dram_tensor("W1HT", [H, TM1], f32, kind="ExternalInput").ap()
    w1st_ap = nc.dram_tensor("W1ST", [H, TM1], f32, kind="ExternalInput").ap()
    w1xt_ap = nc.dram_tensor("W1XT", [TM1, TM1], f32, kind="ExternalInput").ap()
    b1_ap = nc.dram_tensor("B1", [TM1, 1], f32, kind="ExternalInput").ap()
    w2_ap = nc.dram_tensor("W2", [TM1, 1], f32, kind="ExternalInput").ap()
    w2c_ap = nc.dram_tensor("W2C", [TM1, 1], f32, kind="ExternalInput").ap()
    b2_ap = nc.dram_tensor("B2", [128, 1], f32, kind="ExternalInput").ap()
    wiht_ap = nc.dram_tensor("WIHTA", [N + 1, 4 * H], f32,
                             kind="ExternalInput").ap()
    whht_ap = nc.dram_tensor("WHHT", [H, 4 * H], f32, kind="ExternalInput").ap()
    # output h~ in (t, H, b) bf16; host transposes to (b, t, H), scales 0.5
    out_ap = nc.dram_tensor("OUTH", [TM1, H, BL], zdt, kind="ExternalOutput").ap()

    with tile.TileContext(nc) as tc, ExitStack() as ctx:
        consts = ctx.enter_context(tc.tile_pool(name="consts", bufs=1))
        sbig = ctx.enter_context(tc.tile_pool(name="sbig", bufs=1))
        zpa = ctx.enter_context(tc.tile_pool(name="zpa", bufs=3))
        zpd = ctx.enter_context(tc.tile_pool(name="zpd", bufs=3))
        shout = ctx.enter_context(tc.tile_pool(name="shout", bufs=4))
        pp_sb = ctx.enter_context(tc.tile_pool(name="ppsb", bufs=2))

        ps_u = ctx.enter_context(tc.tile_pool(name="psu", bufs=1, space="PSUM"))
        ps_e = ctx.enter_context(tc.tile_pool(name="pse", bufs=1, space="PSUM"))
        ps_xt = ctx.enter_context(tc.tile_pool(name="psxt", bufs=1, space="PSUM"))
        ps_g = ctx.enter_context(tc.tile_pool(name="psg", bufs=1, space="PSUM"))

        # ---- persistent SBUF ----
        def load_cast(ap, p, q, dt2, nm):
            tf = consts.tile([p, q], f32, tag=f"ldf_{nm}")
            nc.sync.dma_start(tf[:], ap[:])
            tb = consts.tile([p, q], dt2, tag=f"ldb_{nm}")
            nc.vector.tensor_copy(tb[:], tf[:])
            return tb, tf

        w1ht, _ = load_cast(w1ht_ap, H, TM1, zdt, "w1ht")
        w1st, _ = load_cast(w1st_ap, H, TM1, zdt, "w1st")
        w1xt, _ = load_cast(w1xt_ap, TM1, TM1, zdt, "w1xt")
        b1sb = consts.tile([TM1, 1], f32)
        nc.sync.dma_start(b1sb[:], b1_ap[:])
        w2sb, _ = load_cast(w2_ap, TM1, 1, zdt, "w2")
        w2csb, _ = load_cast(w2c_ap, TM1, 1, zdt, "w2c")
        b2sb = consts.tile([128, 1], f32)
        nc.sync.dma_start(b2sb[:], b2_ap[:])
        wiht, _ = load_cast(wiht_ap, N + 1, 4 * H, zdt, "wiht")
        whht, _ = load_cast(whht_ap, H, 4 * H, zdt, "whht")

        ident = consts.tile([128, 128], f32)
        make_identity(nc, ident)
        identb = consts.tile([128, 128], zdt)
        nc.vector.tensor_copy(identb[:], ident[:])

        # X_T staged in 8 pieces, cast to bf16 (rhs of the pre matmuls)
        xtb = sbig.tile([TM1, N * BL], zdt)  # (t, n*b) 32KB/p
        NB = N * BL
        for c in range(8):
            stg = pp_sb.tile([TM1, NB // 8], f32, tag="stg")
            nc.sync.dma_start(stg[:], xt_ap[:, c * NB // 8:(c + 1) * NB // 8])
            nc.vector.tensor_copy(xtb[:, c * NB // 8:(c + 1) * NB // 8],
                                  stg[:])

        pre = sbig.tile([TM1, N, BL], zdt)  # (k, n, b) 32KB/p, b1 folded in
        # per-half persistent state (two independent recurrences, b 0:128/128:256)
        HV = {}
        for hf in range(CHUNKS):
            d = {}
            d["exp_e"] = consts.tile([128, N], f32, name=f"expe{hf}", tag=f"expe{hf}")
            d["xtl"] = consts.tile([128, N], zdt, name=f"xtl{hf}", tag=f"xtl{hf}")
            d["s"] = consts.tile([128, 1], f32, name=f"s{hf}", tag=f"s{hf}")
            d["rs"] = consts.tile([128, 1], f32, name=f"rs{hf}", tag=f"rs{hf}")
            d["u"] = consts.tile([TM1, 128], zdt, name=f"u{hf}", tag=f"u{hf}")
            d["hb"] = consts.tile([H, 128], zdt, name=f"hb{hf}", tag=f"hb{hf}")
            d["cb"] = consts.tile([H, 128], zdt, name=f"cb{hf}", tag=f"cb{hf}")
            nc.vector.memset(d["hb"][:], 0.0)
            nc.vector.memset(d["cb"][:], 0.0)
            d["th"] = consts.tile([H, 4 * 128], f32, name=f"th{hf}", tag=f"th{hf}")
            d["A"] = consts.tile([H, 128], f32, name=f"A{hf}", tag=f"A{hf}")
            d["B"] = consts.tile([H, 128], f32, name=f"B{hf}", tag=f"B{hf}")
            d["thc"] = consts.tile([H, 128], f32, name=f"thc{hf}", tag=f"thc{hf}")
            d["xaug"] = consts.tile([N + 1, 128], zdt, name=f"xaug{hf}", tag=f"xaug{hf}")
            nc.vector.memset(d["xaug"][:], 1.0)  # row 64 stays == 1
            HV[hf] = d

        # ---- pre-phase: pre[k, (n b)] = sum_t W1x[k,t] * X_T[t, (n b)] + b1[k]
        pre_f = pre.rearrange("k n b -> k (n b)")
        for c in range(32):
            pre_ps = ps_g.tile([128, 512], f32, tag=f"gp{c % 2}")
            nc.tensor.matmul(pre_ps[:], w1xt[:],
                             xtb[:, c * 512:(c + 1) * 512],
                             start=True, stop=True)
            # fold b1 during the PSUM->SBUF downcast (DVE, per-partition scalar)
            nc.vector.tensor_scalar_add(pre_f[:, c * 512:(c + 1) * 512],
                                        pre_ps[:], b1sb[:])

        # ---- recurrence ----
        def emit_u_c(hf):
            # u_T = 0.5*W1s @ c~ (+ h-part later); scales folded host-side
            d = HV[hf]
            d["u_ps"] = ps_u.tile([TM1, 128], f32, tag=f"ups{hf}",
                                  name=f"ups{hf}")
            nc.tensor.matmul(d["u_ps"][:], w1st[:], d["cb"][:], start=True,
                             stop=False)

        def emit_u_h(hf):
            d = HV[hf]
            nc.tensor.matmul(d["u_ps"][:], w1ht[:], d["hb"][:], start=False,
                             stop=True)
            nc.vector.tensor_copy(d["u"][:], d["u_ps"][:])  # downcast to bf16

        def emit_zwork(hf):
            # next step's z inputs: adds (DVE + one Pool group) then the
            # DVE-path fused tanh-poly customs
            d = HV[hf]
            b0 = hf * 128
            u = d["u"]
            d["za"] = {}
            for gi, (go, gs, eng) in enumerate(AGROUPS):
                u_bc = bass.AP(tensor=u.tensor, offset=u.offset,
                               ap=[u.ap[0], [0, gs], [1, 128]])
                z = zpa.tile([TM1, gs, 128], zdt, tag=f"za{hf}g{gi}",
                             name=f"za{hf}_{gi}")
                v = nc.gpsimd if eng == "pool" else nc.vector
                v.tensor_add(z[:], pre[:, go:go + gs, b0:b0 + 128], u_bc)
                d["za"][gi] = z
            d["zd"] = []
            for g in range(2):
                go = NACT + g * GD
                u_bc = bass.AP(tensor=u.tensor, offset=u.offset,
                               ap=[u.ap[0], [0, GD], [1, 128]])
                z = zpd.tile([TM1, GD, 128], zdt, tag=f"zd{hf}",
                             name=f"zd{hf}_{g}")
                nc.vector._custom_dve(OP, out=z[:],
                                      in0=pre[:, go:go + GD, b0:b0 + 128],
                                      in1=u_bc, s0=POLY_L, s1=POLY_Q)
                d["zd"].append(z)

        def emit_tanh(hf, gi):
            d = HV[hf]
            z = d["za"][gi]
            nc.scalar.activation(z[:], z[:], AF.Tanh)

        def emit_e(hf, gi):
            # e columns for ACT group gi (z stationary, w2 moving)
            d = HV[hf]
            if gi == 0:
                d["e_ps"] = ps_e.tile([128, N], f32, tag=f"emm{hf}",
                                      name=f"eps{hf}")
            e_ps = d["e_ps"]
            z = d["za"][gi]
            go, gs, _ = AGROUPS[gi]
            for j in range(gs):
                nn_ = go + j
                nc.tensor.matmul(e_ps[:, nn_:nn_ + 1], z[:, j, :],
                                 w2sb[:], start=True, stop=True)

        def emit_e_d(hf, g):
            # e columns for DVE group g (poly lead coeff folded into w2c)
            d = HV[hf]
            e_ps = d["e_ps"]
            z = d["zd"][g]
            for j in range(GD):
                nn_ = NACT + g * GD + j
                nc.tensor.matmul(e_ps[:, nn_:nn_ + 1], z[:, j, :],
                                 w2csb[:], start=True, stop=True)

        def part_exp_attn(t, hf):
            d = HV[hf]
            # softmax over n (free dim), fold b_attn2 via ACT bias
            nc.scalar.activation(d["exp_e"][:], d["e_ps"][:], AF.Exp,
                                 bias=b2sb[:], scale=1.0,
                                 accum_out=d["s"][:])
            nc.vector.reciprocal(d["rs"][:], d["s"][:])
            # x~ in (b, n): (exp_e * rs) * x_t  (one fused STT)
            nc.vector.scalar_tensor_tensor(d["xtl"][:], d["exp_e"][:],
                                           d["rs"][:], d["xb"][:],
                                           op0=AL.mult, op1=AL.mult)
            # transpose to (n, b) and land in xaug below the constant 1-row
            xt_ps = ps_xt.tile([128, 128], zdt, tag=f"tp{hf}",
                               name=f"xtps{hf}")
            nc.tensor.transpose(xt_ps[0:N, :], d["xtl"][:], identb[:])
            nc.vector.tensor_copy(d["xaug"][0:N, :], xt_ps[0:N, :])
            # gates psum (H, 4*128) = [i | f | g | o], weights prescaled
            gp = ps_g.tile([H, 4 * 128], f32, tag=f"gp{hf}", name=f"gp{hf}")
            d["gp"] = gp
            for gi in range(4):
                nc.tensor.matmul(gp[:, gi * 128:(gi + 1) * 128],
                                 wiht[:, gi * H:(gi + 1) * H], d["xaug"][:],
                                 start=True, stop=False)
                nc.tensor.matmul(gp[:, gi * 128:(gi + 1) * 128],
                                 whht[:, gi * H:(gi + 1) * H], d["hb"][:],
                                 start=False, stop=True)

        def part_th(hf):
            # th = [tanh(i/2) tanh(f/2) tanh(g) tanh(o/2)] (scales folded)
            # c~ = 0.5*(1+thf)*c~ + (1+thi)*thg, written straight to bf16
            d = HV[hf]
            th = d["th"]
            nc.scalar.activation(th[:], d["gp"][:], AF.Tanh)
            thi, thf_, thg = th[:, 0:128], th[:, 128:256], th[:, 256:384]
            nc.vector.scalar_tensor_tensor(d["A"][:], thf_, 1.0, d["cb"][:],
                                           op0=AL.add, op1=AL.mult)
            nc.vector.scalar_tensor_tensor(d["B"][:], thi, 1.0, thg,
                                           op0=AL.add, op1=AL.mult)
            nc.vector.scalar_tensor_tensor(d["cb"][:], d["A"][:], 0.5,
                                           d["B"][:], op0=AL.mult,
                                           op1=AL.add)
            if d["more"]:
                emit_u_c(hf)

        def part_thc(t, hf, last):
            d = HV[hf]
            tho = d["th"][:, 384:512]
            nc.scalar.activation(d["thc"][:], d["cb"][:], AF.Tanh, scale=0.5)
            # h~ = (1+tho)*tanh(c), written straight to bf16
            nc.vector.scalar_tensor_tensor(d["hb"][:], tho, 1.0, d["thc"][:],
                                           op0=AL.add, op1=AL.mult)
            if not last:
                emit_u_h(hf)
                emit_zwork(hf)
                emit_xb(t + 1, hf)

        def emit_out(t, hf):
            # h~ out in (t, H, b) bf16; host transposes + scales by 0.5
            d = HV[hf]
            b0 = hf * 128
            nc.sync.dma_start(out_ap[t, :, b0:b0 + 128], d["hb"][:])

        def emit_xb(t, hf):
            d = HV[hf]
            b0 = hf * 128
            d["xb"] = shout.tile([128, N], f32, tag=f"xb{hf}",
                                 name=f"xb{hf}")
            nc.sync.dma_start(d["xb"][:], xb_ap[t, b0:b0 + 128, :])

        # Anti-phased emission: the two half-batches run half a step out of
        # phase; each half's tail ACT ops (th/thc) are spliced between the
        # other half's tanh groups so ACT never idles on the tail chains.
        for hf in range(CHUNKS):
            HV[hf]["more"] = True
            emit_u_c(hf)
            emit_u_h(hf)
            emit_zwork(hf)
            emit_xb(0, hf)
        pend = {0: None, 1: None}  # half -> step index with deferred tail
        g1, g3, g2 = AORDER
        for t in range(steps):
            for X in range(CHUNKS):
                Y = 1 - X
                emit_tanh(X, g1)
                emit_e(X, g1)
                if pend[Y] is not None:
                    HV[Y]["more"] = pend[Y] + 1 < steps
                    with tc.high_priority():
                        part_th(Y)
                emit_tanh(X, g3)
                emit_e(X, g3)
                if pend[Y] is not None:
                    with tc.high_priority():
                        part_thc(pend[Y], Y, last=(pend[Y] == steps - 1))
                    emit_out(pend[Y], Y)
                    pend[Y] = None
                emit_tanh(X, g2)
                emit_e(X, g2)
                emit_e_d(X, 0)
                emit_e_d(X, 1)
                with tc.high_priority():
                    part_exp_attn(t, X)
                pend[X] = t
        # drain the last pending tails
        for X in range(CHUNKS):
            if pend[X] is not None:
                HV[X]["more"] = False
                part_th(X)
                part_thc(pend[X], X, last=True)
                emit_out(pend[X], X)
                pend[X] = None

    nc.compile()
    return nc


def _pack_inputs(X, W_attn1, b_attn1, w_attn2, b_attn2, W_ih, W_hh, b_ih,
                 b_hh):
    """Host-side marshalling: shard X, pre-transpose the small weights."""
    f = np.float32
    W_attn1 = np.asarray(W_attn1, f)
    # State is kept as h~=2h, c~=2c and sigmoids are computed via
    # tanh(x/2): fold the needed 0.5 factors into the weights here.
    # gate scale: i,f,o rows get 0.5 (tanh(x/2) trick); g rows keep 1.
    gs = np.concatenate([np.full(H, 0.5, f), np.full(H, 0.5, f),
                         np.ones(H, f), np.full(H, 0.5, f)])  # (4H,)
    w1ht = np.ascontiguousarray(W_attn1[:, :H].T) * 0.5      # h~ = 2h
    w1st = np.ascontiguousarray(W_attn1[:, H:2 * H].T) * 0.5  # c~ = 2c
    w1xt = np.ascontiguousarray(W_attn1[:, 2 * H:].T)
    b1 = np.asarray(b_attn1, f).reshape(TM1, 1)
    w2 = np.asarray(w_attn2, f).reshape(TM1, 1)
    w2c = w2 * np.float32(POLY_C)
    b2 = np.full((128, 1), np.asarray(b_attn2, f).reshape(-1)[0], f)
    b_lstm = ((np.asarray(b_ih, f) + np.asarray(b_hh, f)) * gs).reshape(
        1, 4 * H)
    wihta = np.concatenate(
        [np.ascontiguousarray(np.asarray(W_ih, f).T) * gs, b_lstm], axis=0)
    whht = np.ascontiguousarray(np.asarray(W_hh, f).T) * gs * 0.5  # h~ = 2h
    X = np.asarray(X, f)
    maps = []
    for i in range(NCORES):
        Xc = X[i * BL:(i + 1) * BL]                        # (BL, T, N)
        xt = np.ascontiguousarray(Xc.transpose(1, 2, 0)).reshape(TM1, N * BL)
        xb = np.ascontiguousarray(Xc.transpose(1, 0, 2))   # (T, BL, N)
        maps.append({
            "XT": xt, "XB": xb,
            "W1HT": w1ht, "W1ST": w1st, "W1XT": w1xt,
            "B1": b1, "W2": w2, "W2C": w2c, "B2": b2,
            "WIHTA": wihta, "WHHT": whht,
        })
    return maps


def _get_nc():
    if "nc" not in _cache:
        steps = int(os.environ.get("KERNEL_STEPS", TM1))
        _cache["nc"] = _build(steps)
    return _cache["nc"]


def run(trace=False, **inputs):
    from concourse.bass_utils import run_bass_kernel_spmd
    nc = _get_nc()
    in_maps = _pack_inputs(**inputs)
    res = run_bass_kernel_spmd(nc, in_maps, core_ids=list(range(NCORES)),
                               trace=trace)
    # OUTH is (T, H, BL) bf16 h~ per core; out = 0.5 * transpose -> (B, T, H)
    outs = []
    for i in range(NCORES):
        oh = np.asarray(res.results[i]["OUTH"]).astype(np.float32)
        outs.append(0.5 * np.ascontiguousarray(oh.transpose(2, 0, 1)))
    out = np.concatenate(outs, axis=0)
    return out, res


def kernel(**inputs) -> np.ndarray:
    out, _ = run(trace=False, **inputs)
    return out


# revision 6
# speedup vs baseline: 1.4719x; 1.4719x over previous
"""Trainium2 Bass kernel for nn_Encoder (input-attention LSTM encoder).

Contract: kernel(**inputs) takes the FULL unsharded inputs (numpy) and
returns the FULL output (B, T-1, H) float32.  Internally shards the batch
across 8 NeuronCores (pure data parallel), runs a Bass/Tile kernel per
core, and concatenates the per-core outputs.

Math per timestep t (see reference):
    z     = tanh(pre_x + (h @ W1_h.T)[:,None,:] + (c @ W1_s.T)[:,None,:])
    e     = z @ w_attn2 + b_attn2            # (B, N)
    alpha = softmax_n(e)
    x~    = alpha * x_t
    gates = x~ @ W_ih.T + h @ W_hh.T + b     # (B, 4H)
    LSTM update (i, f, g, o) -> h, c

Engine split of the z/e path (2M elems per step per core, the bottleneck):
  - n-groups g1,g3 ([0:6), [26:46)): DVE tensor_tensor add (pre+u, 2x bf16),
    exact ACT tanh.  Group g2 ([6:26)): the add runs on GpSimd (Pool) to
    offload DVE; its tanh runs last so Pool latency hides.
  - n in [46, 64): ONE fused custom DVE op per group computes
    q = ((clip(pre+u,+-L)^2 - L^2)^2 + Q) * clip(pre+u,+-L); the deg-5 lead
    coeff C is folded into the w_attn2 column those n's use (max |C*q-tanh|
    = 3.3e-2, washes out through softmax+LSTM; validated end to end).
State h~/c~ lives only in bf16 (validated numerically), which kills the
state downcasts; h~ is DMA'd out per step in (t,H,b) bf16 layout and the
host does the final transpose + 0.5 scale + f32 cast.
"""

import os

import numpy as np

B, TM1, N, H = 2048, 128, 64, 128
NCORES = 8
BL = B // NCORES  # 256 batch rows per core
CHUNKS = 2        # BL / 128

NACT = 50                 # n's on the exact ACT-tanh path
NDVE = N - NACT           # n's on the fused DVE polynomial path
GD = NDVE // 2            # two DVE-poly groups
# (offset, size, add-engine); tanh/e processed in order g1, g3, g2
AGROUPS = ((0, 6, "dve"), (6, 22, "dve"), (28, 22, "dve"))
AORDER = (0, 2, 1)

# deg-5 fused-tanh fit: P(a) = POLY_C * ac * ((ac^2 - L^2)^2 + Q), ac=clip(a,+-L)
POLY_L = 2.65
POLY_Q = 37.43353826998534
POLY_C = 0.0103122888175463

_cache = {}


def _register_tanh_op():
    """Register the fused add+clip+deg5-tanh custom DVE op (official
    extension point: concourse.dve_ops.OPS)."""
    from concourse import dve_ops
    from concourse.dve_spec import Spec, Src0, Src1, C0, C1, Zero, minn, maxx, lower
    from concourse.dve_uop import DveOpSpec

    name = "TANH5_FUSED_ADD_ANT"
    for o in dve_ops.OPS:
        if o.name == name:
            return o
    a = Src0 + Src1
    ac = maxx(minn(a, C0), Zero - C0)
    s = ac * ac
    g = s - C0 * C0
    body = (g * g + C1) * ac

    def ref(in0, in1, s0, s1, imm2):
        aa = in0 + in1
        acc = np.clip(aa, -s0, s0)
        ss = acc * acc
        return ((((ss - s0 * s0) ** 2) + s1) * acc).astype(np.float32)

    spec = Spec(body=body, reference=ref)
    row = dve_ops._CUSTOM_DVE_ROW_BASE + len(dve_ops.OPS)
    shas = {}
    for ver in ("v3", "v4"):
        uops = lower(spec, ver=ver)
        shas[ver] = DveOpSpec(name=name, opcode=row, uops=uops, rd1_en=True).sha(ver)
    op = dve_ops.DveOp(name, spec, subdim=False, uops_sha=shas)
    dve_ops.OPS.append(op)
    dve_ops.CUSTOM_DVE_SPECS[name] = spec
    dve_ops._SUB_OPCODE_FOR_NAME[name] = row
    return op


def _build(steps=TM1):
    """Trace + compile the per-core Bass kernel. Returns the Bacc object."""
    from contextlib import ExitStack

    import concourse.bass as bass
    import concourse.tile as tile
    from concourse import bacc, mybir
    from concourse.masks import make_identity

    OP = _register_tanh_op()

    f32 = mybir.dt.float32
    zdt = mybir.dt.bfloat16  # dtype of the z path + weights + state
    AF = mybir.ActivationFunctionType
    AL = mybir.AluOpType

    nc = bacc.Bacc("TRN2", target_bir_lowering=False, debug=False,
                   num_devices=NCORES)

    # X_T: (T, N*BL)  — rhs for the pre-matmul (t on partitions)
    # X_B: (TM1, BL, N) — per-step x_t in (b, n) layout, streamed from DRAM
    xt_ap = nc.dram_tensor("XT", [TM1, N * BL], f32, kind="ExternalInput").ap()
    xb_ap = nc.dram_tensor("XB", [TM1, BL, N], f32, kind="ExternalInput").ap()
    w1ht_ap = nc.dram_tensor("W1HT", [H, TM1], f32, kind="ExternalInput").ap()
    w1st_ap = nc.dram_tensor("W1ST", [H, TM1], f32, kind="ExternalInput").ap()
    w1xt_ap = nc.dram_tensor("W1XT", [TM1, TM1], f32, kind="ExternalInput").ap()
    b1_ap = nc.dram_tensor("B1", [TM1, 1], f32, kind="ExternalInput").ap()
    w2_ap = nc.dram_tensor("W2", [TM1, 1], f32, kind="ExternalInput").ap()
    w2c_ap = nc.dram_tensor("W2C", [TM1, 1], f32, kind="ExternalInput").ap()
    b2_ap = nc.dram_tensor("B2", [128, 1], f32, kind="ExternalInput").ap()
    wiht_ap = nc.dram_tensor("WIHTA", [N + 1, 4 * H], f32,
                             kind="ExternalInput").ap()
    whht_ap = nc.dram_tensor("WHHT", [H, 4 * H], f32, kind="ExternalInput").ap()
    # output h~ in (t, H, b) bf16; host transposes to (b, t, H), scales 0.5
    out_ap = nc.dram_tensor("OUTH", [TM1, H, BL], zdt, kind="ExternalOutput").ap()

    with tile.TileContext(nc) as tc, ExitStack() as ctx:
        consts = ctx.enter_context(tc.tile_pool(name="consts", bufs=1))
        sbig = ctx.enter_context(tc.tile_pool(name="sbig", bufs=1))
        zpa = ctx.enter_context(tc.tile_pool(name="zpa", bufs=3))
        zpd = ctx.enter_context(tc.tile_pool(name="zpd", bufs=3))
        shout = ctx.enter_context(tc.tile_pool(name="shout", bufs=4))
        pp_sb = ctx.enter_context(tc.tile_pool(name="ppsb", bufs=2))

        ps_u = ctx.enter_context(tc.tile_pool(name="psu", bufs=1, space="PSUM"))
        ps_e = ctx.enter_context(tc.tile_pool(name="pse", bufs=1, space="PSUM"))
        ps_xt = ctx.enter_context(tc.tile_pool(name="psxt", bufs=1, space="PSUM"))
        ps_g = ctx.enter_context(tc.tile_pool(name="psg", bufs=1, space="PSUM"))

        # ---- persistent SBUF ----
        def load_cast(ap, p, q, dt2, nm):
            tf = consts.tile([p, q], f32, tag=f"ldf_{nm}")
            nc.sync.dma_start(tf[:], ap[:])
            tb = consts.tile([p, q], dt2, tag=f"ldb_{nm}")
            nc.vector.tensor_copy(tb[:], tf[:])
            return tb, tf

        w1ht, _ = load_cast(w1ht_ap, H, TM1, zdt, "w1ht")
        w1st, _ = load_cast(w1st_ap, H, TM1, zdt, "w1st")
        w1xt, _ = load_cast(w1xt_ap, TM1, TM1, zdt, "w1xt")
        b1sb = consts.tile([TM1, 1], f32)
        nc.sync.dma_start(b1sb[:], b1_ap[:])
        w2sb, _ = load_cast(w2_ap, TM1, 1, zdt, "w2")
        w2csb, _ = load_cast(w2c_ap, TM1, 1, zdt, "w2c")
        b2sb = consts.tile([128, 1], f32)
        nc.sync.dma_start(b2sb[:], b2_ap[:])
        wiht, _ = load_cast(wiht_ap, N + 1, 4 * H, zdt, "wiht")
        whht, _ = load_cast(whht_ap, H, 4 * H, zdt, "whht")

        ident = consts.tile([128, 128], f32)
        make_identity(nc, ident)
        identb = consts.tile([128, 128], zdt)
        nc.vector.tensor_copy(identb[:], ident[:])

        # X_T staged in 8 pieces, cast to bf16 (rhs of the pre matmuls)
        xtb = sbig.tile([TM1, N * BL], zdt)  # (t, n*b) 32KB/p
        NB = N * BL
        for c in range(8):
            stg = pp_sb.tile([TM1, NB // 8], f32, tag="stg")
            nc.sync.dma_start(stg[:], xt_ap[:, c * NB // 8:(c + 1) * NB // 8])
            nc.vector.tensor_copy(xtb[:, c * NB // 8:(c + 1) * NB // 8],
                                  stg[:])

        pre = sbig.tile([TM1, N, BL], zdt)  # (k, n, b) 32KB/p, b1 folded in
        # per-half persistent state (two independent recurrences, b 0:128/128:256)
        HV = {}
        for hf in range(CHUNKS):
            d = {}
            d["exp_e"] = consts.tile([128, N], f32, name=f"expe{hf}", tag=f"expe{hf}")
            d["xtl"] = consts.tile([128, N], zdt, name=f"xtl{hf}", tag=f"xtl{hf}")
            d["s"] = consts.tile([128, 1], f32, name=f"s{hf}", tag=f"s{hf}")
            d["rs"] = consts.tile([128, 1], f32, name=f"rs{hf}", tag=f"rs{hf}")
            d["u"] = consts.tile([TM1, 128], zdt, name=f"u{hf}", tag=f"u{hf}")
            d["hb"] = consts.tile([H, 128], zdt, name=f"hb{hf}", tag=f"hb{hf}")
            d["cb"] = consts.tile([H, 128], zdt, name=f"cb{hf}", tag=f"cb{hf}")
            nc.vector.memset(d["hb"][:], 0.0)
            nc.vector.memset(d["cb"][:], 0.0)
            d["th"] = consts.tile([H, 4 * 128], f32, name=f"th{hf}", tag=f"th{hf}")
            d["A"] = consts.tile([H, 128], f32, name=f"A{hf}", tag=f"A{hf}")
            d["B"] = consts.tile([H, 128], f32, name=f"B{hf}", tag=f"B{hf}")
            d["thc"] = consts.tile([H, 128], f32, name=f"thc{hf}", tag=f"thc{hf}")
            d["xaug"] = consts.tile([N + 1, 128], zdt, name=f"xaug{hf}", tag=f"xaug{hf}")
            nc.vector.memset(d["xaug"][:], 1.0)  # row 64 stays == 1
            HV[hf] = d

        # ---- pre-phase: pre[k, (n b)] = sum_t W1x[k,t] * X_T[t, (n b)] + b1[k]
        pre_f = pre.rearrange("k n b -> k (n b)")
        for c in range(32):
            pre_ps = ps_g.tile([128, 512], f32, tag=f"gp{c % 2}")
            nc.tensor.matmul(pre_ps[:], w1xt[:],
                             xtb[:, c * 512:(c + 1) * 512],
                             start=True, stop=True)
            # fold b1 during the PSUM->SBUF downcast (DVE, per-partition scalar)
            nc.vector.tensor_scalar_add(pre_f[:, c * 512:(c + 1) * 512],
                                        pre_ps[:], b1sb[:])

        # ---- recurrence ----
        def emit_u_c(hf):
            # u_T = 0.5*W1s @ c~ (+ h-part later); scales folded host-side
            d = HV[hf]
            d["u_ps"] = ps_u.tile([TM1, 128], f32, tag=f"ups{hf}",
                                  name=f"ups{hf}")
            nc.tensor.matmul(d["u_ps"][:], w1st[:], d["cb"][:], start=True,
                             stop=False)

        def emit_u_h(hf):
            d = HV[hf]
            nc.tensor.matmul(d["u_ps"][:], w1ht[:], d["hb"][:], start=False,
                             stop=True)
            nc.scalar.copy(d["u"][:], d["u_ps"][:])  # downcast to bf16 (ACT)

        def emit_zwork(hf):
            # next step's z inputs: adds (DVE + one Pool group) then the
            # DVE-path fused tanh-poly customs
            d = HV[hf]
            b0 = hf * 128
            u = d["u"]
            d["za"] = {}
            for gi, (go, gs, eng) in enumerate(AGROUPS):
                u_bc = bass.AP(tensor=u.tensor, offset=u.offset,
                               ap=[u.ap[0], [0, gs], [1, 128]])
                z = zpa.tile([TM1, gs, 128], zdt, tag=f"za{hf}g{gi}",
                             name=f"za{hf}_{gi}")
                v = nc.gpsimd if eng == "pool" else nc.vector
                v.tensor_add(z[:], pre[:, go:go + gs, b0:b0 + 128], u_bc)
                d["za"][gi] = z
            d["zd"] = []
            for g in range(2):
                go = NACT + g * GD
                u_bc = bass.AP(tensor=u.tensor, offset=u.offset,
                               ap=[u.ap[0], [0, GD], [1, 128]])
                z = zpd.tile([TM1, GD, 128], zdt, tag=f"zd{hf}",
                             name=f"zd{hf}_{g}")
                nc.vector._custom_dve(OP, out=z[:],
                                      in0=pre[:, go:go + GD, b0:b0 + 128],
                                      in1=u_bc, s0=POLY_L, s1=POLY_Q)
                d["zd"].append(z)

        def emit_tanh(hf, gi):
            d = HV[hf]
            z = d["za"][gi]
            nc.scalar.activation(z[:], z[:], AF.Tanh)

        def emit_e(hf, gi):
            # e columns for ACT group gi (z stationary, w2 moving)
            d = HV[hf]
            if gi == 0:
                d["e_ps"] = ps_e.tile([128, N], f32, tag=f"emm{hf}",
                                      name=f"eps{hf}")
            e_ps = d["e_ps"]
            z = d["za"][gi]
            go, gs, _ = AGROUPS[gi]
            for j in range(gs):
                nn_ = go + j
                nc.tensor.matmul(e_ps[:, nn_:nn_ + 1], z[:, j, :],
                                 w2sb[:], start=True, stop=True)

        def emit_e_d(hf, g):
            # e columns for DVE group g (poly lead coeff folded into w2c)
            d = HV[hf]
            e_ps = d["e_ps"]
            z = d["zd"][g]
            for j in range(GD):
                nn_ = NACT + g * GD + j
                nc.tensor.matmul(e_ps[:, nn_:nn_ + 1], z[:, j, :],
                                 w2csb[:], start=True, stop=True)

        def part_exp_attn(t, hf):
            d = HV[hf]
            # softmax over n (free dim), fold b_attn2 via ACT bias
            nc.scalar.activation(d["exp_e"][:], d["e_ps"][:], AF.Exp,
                                 bias=b2sb[:], scale=1.0,
                                 accum_out=d["s"][:])
            nc.vector.reciprocal(d["rs"][:], d["s"][:])
            # x~ in (b, n): (exp_e * rs) * x_t  (one fused STT)
            nc.vector.scalar_tensor_tensor(d["xtl"][:], d["exp_e"][:],
                                           d["rs"][:], d["xb"][:],
                                           op0=AL.mult, op1=AL.mult)
            # transpose to (n, b) and land in xaug below the constant 1-row
            xt_ps = ps_xt.tile([128, 128], zdt, tag=f"tp{hf}",
                               name=f"xtps{hf}")
            nc.tensor.transpose(xt_ps[0:N, :], d["xtl"][:], identb[:])
            nc.vector.tensor_copy(d["xaug"][0:N, :], xt_ps[0:N, :])
            # gates psum (H, 4*128) = [i | f | g | o], weights prescaled
            gp = ps_g.tile([H, 4 * 128], f32, tag=f"gp{hf}", name=f"gp{hf}")
            d["gp"] = gp
            for gi in range(4):
                nc.tensor.matmul(gp[:, gi * 128:(gi + 1) * 128],
                                 wiht[:, gi * H:(gi + 1) * H], d["xaug"][:],
                                 start=True, stop=False)
                nc.tensor.matmul(gp[:, gi * 128:(gi + 1) * 128],
                                 whht[:, gi * H:(gi + 1) * H], d["hb"][:],
                                 start=False, stop=True)

        def part_th(hf):
            # th = [tanh(i/2) tanh(f/2) tanh(g) tanh(o/2)] (scales folded)
            # c~ = 0.5*(1+thf)*c~ + (1+thi)*thg, written straight to bf16
            d = HV[hf]
            th = d["th"]
            nc.scalar.activation(th[:], d["gp"][:], AF.Tanh)
            thi, thf_, thg = th[:, 0:128], th[:, 128:256], th[:, 256:384]
            nc.vector.scalar_tensor_tensor(d["A"][:], thf_, 1.0, d["cb"][:],
                                           op0=AL.add, op1=AL.mult)
            nc.vector.scalar_tensor_tensor(d["B"][:], thi, 1.0, thg,
                                           op0=AL.add, op1=AL.mult)
            nc.vector.scalar_tensor_tensor(d["cb"][:], d["A"][:], 0.5,
                                           d["B"][:], op0=AL.mult,
                                           op1=AL.add)
            if d["more"]:
                emit_u_c(hf)

        def part_thc(t, hf, last):
            d = HV[hf]
            tho = d["th"][:, 384:512]
            nc.scalar.activation(d["thc"][:], d["cb"][:], AF.Tanh, scale=0.5)
            # h~ = (1+tho)*tanh(c), written straight to bf16
            nc.vector.scalar_tensor_tensor(d["hb"][:], tho, 1.0, d["thc"][:],
                                           op0=AL.add, op1=AL.mult)
            if not last:
                emit_u_h(hf)
                emit_zwork(hf)
                emit_xb(t + 1, hf)

        def emit_out(t, hf):
            # h~ out in (t, H, b) bf16; host transposes + scales by 0.5
            d = HV[hf]
            b0 = hf * 128
            nc.sync.dma_start(out_ap[t, :, b0:b0 + 128], d["hb"][:])

        def emit_xb(t, hf):
            d = HV[hf]
            b0 = hf * 128
            d["xb"] = shout.tile([128, N], f32, tag=f"xb{hf}",
                                 name=f"xb{hf}")
            nc.sync.dma_start(d["xb"][:], xb_ap[t, b0:b0 + 128, :])

        # Anti-phased emission: the two half-batches run half a step out of
        # phase; each half's tail ACT ops (th/thc) are spliced between the
        # other half's tanh groups so ACT never idles on the tail chains.
        for hf in range(CHUNKS):
            HV[hf]["more"] = True
            emit_u_c(hf)
            emit_u_h(hf)
            emit_zwork(hf)
            emit_xb(0, hf)
        pend = {0: None, 1: None}  # half -> step index with deferred tail
        g1, g3, g2 = AORDER
        for t in range(steps):
            for X in range(CHUNKS):
                Y = 1 - X
                emit_tanh(X, g1)
                emit_e(X, g1)
                if pend[Y] is not None:
                    HV[Y]["more"] = pend[Y] + 1 < steps
                    with tc.high_priority():
                        part_th(Y)
                emit_tanh(X, g3)
                emit_e(X, g3)
                if pend[Y] is not None:
                    with tc.high_priority():
                        part_thc(pend[Y], Y, last=(pend[Y] == steps - 1))
                    emit_out(pend[Y], Y)
                    pend[Y] = None
                emit_tanh(X, g2)
                emit_e(X, g2)
                emit_e_d(X, 0)
                emit_e_d(X, 1)
                with tc.high_priority():
                    part_exp_attn(t, X)
                pend[X] = t
        # drain the last pending tails
        for X in range(CHUNKS):
            if pend[X] is not None:
                HV[X]["more"] = False
                part_th(X)
                part_thc(pend[X], X, last=True)
                emit_out(pend[X], X)
                pend[X] = None

    nc.compile()
    return nc


def _pack_inputs(X, W_attn1, b_attn1, w_attn2, b_attn2, W_ih, W_hh, b_ih,
                 b_hh):
    """Host-side marshalling: shard X, pre-transpose the small weights."""
    f = np.float32
    W_attn1 = np.asarray(W_attn1, f)
    # State is kept as h~=2h, c~=2c and sigmoids are computed via
    # tanh(x/2): fold the needed 0.5 factors into the weights here.
    # gate scale: i,f,o rows get 0.5 (tanh(x/2) trick); g rows keep 1.
    gs = np.concatenate([np.full(H, 0.5, f), np.full(H, 0.5, f),
                         np.ones(H, f), np.full(H, 0.5, f)])  # (4H,)
    w1ht = np.ascontiguousarray(W_attn1[:, :H].T) * 0.5      # h~ = 2h
    w1st = np.ascontiguousarray(W_attn1[:, H:2 * H].T) * 0.5  # c~ = 2c
    w1xt = np.ascontiguousarray(W_attn1[:, 2 * H:].T)
    b1 = np.asarray(b_attn1, f).reshape(TM1, 1)
    w2 = np.asarray(w_attn2, f).reshape(TM1, 1)
    w2c = w2 * np.float32(POLY_C)
    b2 = np.full((128, 1), np.asarray(b_attn2, f).reshape(-1)[0], f)
    b_lstm = ((np.asarray(b_ih, f) + np.asarray(b_hh, f)) * gs).reshape(
        1, 4 * H)
    wihta = np.concatenate(
        [np.ascontiguousarray(np.asarray(W_ih, f).T) * gs, b_lstm], axis=0)
    whht = np.ascontiguousarray(np.asarray(W_hh, f).T) * gs * 0.5  # h~ = 2h
    X = np.asarray(X, f)
    maps = []
    for i in range(NCORES):
        Xc = X[i * BL:(i + 1) * BL]                        # (BL, T, N)
        xt = np.ascontiguousarray(Xc.transpose(1, 2, 0)).reshape(TM1, N * BL)
        xb = np.ascontiguousarray(Xc.transpose(1, 0, 2))   # (T, BL, N)
        maps.append({
            "XT": xt, "XB": xb,
            "W1HT": w1ht, "W1ST": w1st, "W1XT": w1xt,
            "B1": b1, "W2": w2, "W2C": w2c, "B2": b2,
            "WIHTA": wihta, "WHHT": whht,
        })
    return maps


def _get_nc():
    if "nc" not in _cache:
        steps = int(os.environ.get("KERNEL_STEPS", TM1))
        _cache["nc"] = _build(steps)
    return _cache["nc"]


def run(trace=False, **inputs):
    from concourse.bass_utils import run_bass_kernel_spmd
    nc = _get_nc()
    in_maps = _pack_inputs(**inputs)
    res = run_bass_kernel_spmd(nc, in_maps, core_ids=list(range(NCORES)),
                               trace=trace)
    # OUTH is (T, H, BL) bf16 h~ per core; out = 0.5 * transpose -> (B, T, H)
    outs = []
    for i in range(NCORES):
        oh = np.asarray(res.results[i]["OUTH"]).astype(np.float32)
        outs.append(0.5 * np.ascontiguousarray(oh.transpose(2, 0, 1)))
    out = np.concatenate(outs, axis=0)
    return out, res


def kernel(**inputs) -> np.ndarray:
    out, _ = run(trace=False, **inputs)
    return out


# revision 8
# speedup vs baseline: 1.5778x; 1.0719x over previous
"""Trainium2 Bass kernel for nn_Encoder (input-attention LSTM encoder).

Contract: kernel(**inputs) takes the FULL unsharded inputs (numpy) and
returns the FULL output (B, T-1, H) float32.  Internally shards the batch
across 8 NeuronCores (pure data parallel), runs a Bass/Tile kernel per
core, and concatenates the per-core outputs.

Math per timestep t (see reference):
    z     = tanh(pre_x + (h @ W1_h.T)[:,None,:] + (c @ W1_s.T)[:,None,:])
    e     = z @ w_attn2 + b_attn2            # (B, N)
    alpha = softmax_n(e)
    x~    = alpha * x_t
    gates = x~ @ W_ih.T + h @ W_hh.T + b     # (B, 4H)
    LSTM update (i, f, g, o) -> h, c
with pre_x = einsum('bnt,kt->bnk', X.swap(1,2), W1_x) + b_attn1 fixed over t.

Per-core layouts:
    state h_T, c_T             : (H=128 partitions, B=256 free)  f32
    pre, z                     : (k=T=128 partitions, n*256+b free)
    e, exp(e), x_tilde         : (b=128 partitions x 2 chunks, n) via
                                 stationary-z matmuls (PE transposes free)
    gates                      : (H partitions, B free), biases folded in
"""

import os
import sys
import time

import numpy as np

B, TM1, N, H = 2048, 128, 64, 128
NCORES = 8
BL = B // NCORES  # 256 batch rows per core
CHUNKS = 2        # BL / 128
GSIZES = (4, 16, 16, 16, 8, 4)  # n's per z-group (small first+last)
GOFF = (0, 4, 20, 36, 52, 60)
NG = len(GSIZES)
NGMAX = max(GSIZES)
CUSTOM_GROUPS = (4, 5)   # z-groups computed by the fused DVE tanh-poly op

# deg-5 fused-tanh fit: P(a) = POLY_C * ac * ((ac^2-L^2)^2 + Q), ac=clip(a,+-L)
POLY_L = 2.65
POLY_Q = 37.43353826998534
POLY_C = 0.0103122888175463

_cache = {}


def _register_tanh_op():
    from concourse import dve_ops
    from concourse.dve_spec import Spec, Src0, Src1, C0, C1, Zero, minn, maxx, lower
    from concourse.dve_uop import DveOpSpec

    name = "TANH5_FUSED_ADD_ANT"
    for o in dve_ops.OPS:
        if o.name == name:
            return o
    a = Src0 + Src1
    ac = maxx(minn(a, C0), Zero - C0)
    s = ac * ac
    g = s - C0 * C0
    body = (g * g + C1) * ac

    def ref(in0, in1, s0, s1, imm2):
        aa = in0 + in1
        acc = np.clip(aa, -s0, s0)
        ss = acc * acc
        return ((((ss - s0 * s0) ** 2) + s1) * acc).astype(np.float32)

    spec = Spec(body=body, reference=ref)
    row = dve_ops._CUSTOM_DVE_ROW_BASE + len(dve_ops.OPS)
    shas = {}
    for ver in ("v3", "v4"):
        uops = lower(spec, ver=ver)
        shas[ver] = DveOpSpec(name=name, opcode=row, uops=uops, rd1_en=True).sha(ver)
    op = dve_ops.DveOp(name, spec, subdim=False, uops_sha=shas)
    dve_ops.OPS.append(op)
    dve_ops.CUSTOM_DVE_SPECS[name] = spec
    dve_ops._SUB_OPCODE_FOR_NAME[name] = row
    return op


def _build(steps=TM1):
    """Trace + compile the per-core Bass kernel. Returns the Bacc object."""
    from contextlib import ExitStack

    import concourse.bass as bass
    import concourse.tile as tile
    from concourse import bacc, mybir
    from concourse.masks import make_identity

    OP = _register_tanh_op()

    f32 = mybir.dt.float32
    zdt = mybir.dt.bfloat16  # dtype of the pre/z tanh path
    AF = mybir.ActivationFunctionType

    nc = bacc.Bacc("TRN2", target_bir_lowering=False, debug=False,
                   num_devices=NCORES)

    # X_T: (T, N, BL)  — rhs for the pre-matmul (t on partitions)
    # X_TN: (N, TM1, BL) — per-step x_t in (n, b) layout, streamed from DRAM
    xt_ap = nc.dram_tensor("XT", [TM1, N * BL], f32, kind="ExternalInput").ap()
    xtn_ap = nc.dram_tensor("XTN", [N, TM1, BL], f32,
                            kind="ExternalInput").ap()
    w1ht_ap = nc.dram_tensor("W1HT", [H, TM1], f32, kind="ExternalInput").ap()
    w1st_ap = nc.dram_tensor("W1ST", [H, TM1], f32, kind="ExternalInput").ap()
    w1xt_ap = nc.dram_tensor("W1XT", [TM1, TM1], f32, kind="ExternalInput").ap()
    b1_ap = nc.dram_tensor("B1", [TM1, 1], f32, kind="ExternalInput").ap()
    w2_ap = nc.dram_tensor("W2", [TM1, 1], f32, kind="ExternalInput").ap()
    w2c_ap = nc.dram_tensor("W2C", [TM1, 1], f32, kind="ExternalInput").ap()
    b2_ap = nc.dram_tensor("B2", [128, 1], f32, kind="ExternalInput").ap()
    wiht_ap = nc.dram_tensor("WIHTA", [N + 1, 4 * H], f32,
                             kind="ExternalInput").ap()
    whht_ap = nc.dram_tensor("WHHT", [H, 4 * H], f32, kind="ExternalInput").ap()
    out_ap = nc.dram_tensor("OUT", [BL, TM1, H], f32, kind="ExternalOutput").ap()

    with tile.TileContext(nc) as tc, ExitStack() as ctx:
        consts = ctx.enter_context(tc.tile_pool(name="consts", bufs=1))
        sbig = ctx.enter_context(tc.tile_pool(name="sbig", bufs=1))
        zpre_pool = ctx.enter_context(tc.tile_pool(name="zpre", bufs=10))
        ssm = ctx.enter_context(tc.tile_pool(name="ssm", bufs=2))
        shout = ctx.enter_context(tc.tile_pool(name="shout", bufs=4))
        pp_sb = ctx.enter_context(tc.tile_pool(name="ppsb", bufs=3))

        ps_u = ctx.enter_context(tc.tile_pool(name="psu", bufs=1, space="PSUM"))
        ps_e = ctx.enter_context(tc.tile_pool(name="pse", bufs=1, space="PSUM"))
        ps_xt = ctx.enter_context(tc.tile_pool(name="psxt", bufs=1, space="PSUM"))
        ps_g = ctx.enter_context(tc.tile_pool(name="psg", bufs=1, space="PSUM"))
        ps_h = ps_xt  # h-transpose timeshares the tp{hf} banks

        # ---- persistent SBUF ----
        def load_cast(ap, p, q, dt2, nm):
            tf = consts.tile([p, q], f32, tag=f"ldf_{nm}")
            nc.sync.dma_start(tf[:], ap[:])
            tb = consts.tile([p, q], dt2, tag=f"ldb_{nm}")
            nc.vector.tensor_copy(tb[:], tf[:])
            return tb, tf

        w1ht, w1htF = load_cast(w1ht_ap, H, TM1, zdt, "w1ht")
        w1st, _ = load_cast(w1st_ap, H, TM1, zdt, "w1st")
        w1xt, _ = load_cast(w1xt_ap, TM1, TM1, zdt, "w1xt")
        b1sb = consts.tile([TM1, 1], f32)
        nc.sync.dma_start(b1sb[:], b1_ap[:])
        w2sb, _ = load_cast(w2_ap, TM1, 1, zdt, "w2")
        w2csb, _ = load_cast(w2c_ap, TM1, 1, zdt, "w2c")
        b2sb = consts.tile([128, 1], f32)
        nc.sync.dma_start(b2sb[:], b2_ap[:])
        wiht, _ = load_cast(wiht_ap, N + 1, 4 * H, zdt, "wiht")
        whht, _ = load_cast(whht_ap, H, 4 * H, zdt, "whht")

        ident = consts.tile([128, 128], f32)
        make_identity(nc, ident)
        identb = consts.tile([128, 128], zdt)
        nc.vector.tensor_copy(identb[:], ident[:])

        # X_T staged in 4 pieces, cast to bf16 (rhs of the pre matmuls)
        xtb = sbig.tile([TM1, N * BL], zdt)  # (t, n*b) 32KB/p
        NB = N * BL
        for c in range(8):
            stg = pp_sb.tile([TM1, NB // 8], f32, tag="stg")
            nc.sync.dma_start(stg[:], xt_ap[:, c * NB // 8:(c + 1) * NB // 8])
            nc.vector.tensor_copy(xtb[:, c * NB // 8:(c + 1) * NB // 8],
                                  stg[:])

        pre = sbig.tile([TM1, N, BL], zdt)  # (k, n, b) 32KB/p
        # per-half persistent state (two independent recurrences, b 0:128 / 128:256)
        HV = {}
        for hf in range(CHUNKS):
            d = {}
            d["exp_e"] = consts.tile([128, N], f32, name=f"expe{hf}", tag=f"expe{hf}")
            d["xtil"] = consts.tile([128, N], zdt, name=f"xtil{hf}", tag=f"xtil{hf}")
            d["s"] = consts.tile([128, 1], f32, name=f"s{hf}", tag=f"s{hf}")
            d["rs"] = consts.tile([128, 1], f32, name=f"rs{hf}", tag=f"rs{hf}")
            d["u"] = consts.tile([TM1, 128], zdt, name=f"u{hf}", tag=f"u{hf}")
            d["h"] = consts.tile([H, 128], f32, name=f"h{hf}", tag=f"h{hf}")     # h~ = 2h
            d["c"] = consts.tile([H, 128], f32, name=f"c{hf}", tag=f"c{hf}")     # c~ = 2c
            d["hb"] = consts.tile([H, 128], zdt, name=f"hb{hf}", tag=f"hb{hf}")
            d["cb"] = consts.tile([H, 128], zdt, name=f"cb{hf}", tag=f"cb{hf}")
            nc.vector.memset(d["h"][:], 0.0)
            nc.vector.memset(d["c"][:], 0.0)
            nc.vector.memset(d["hb"][:], 0.0)
            nc.vector.memset(d["cb"][:], 0.0)
            d["th"] = consts.tile([H, 4 * 128], f32, name=f"th{hf}", tag=f"th{hf}")
            d["A"] = consts.tile([H, 128], f32, name=f"A{hf}", tag=f"A{hf}")
            d["B"] = consts.tile([H, 128], f32, name=f"B{hf}", tag=f"B{hf}")
            d["thc"] = consts.tile([H, 128], f32, name=f"thc{hf}", tag=f"thc{hf}")
            d["xaug"] = consts.tile([N + 1, 128], zdt, name=f"xaug{hf}", tag=f"xaug{hf}")
            nc.vector.memset(d["xaug"][:], 1.0)  # row 64 stays == 1
            HV[hf] = d

        # ---- pre-phase: pre[k, (n b)] = sum_t W1x[k,t] * X_T[t, (n b)] ----
        # (b_attn1 is folded into the tanh bias each step)
        pre_f = pre.rearrange("k n b -> k (n b)")
        for c in range(32):
            pre_ps = ps_g.tile([128, 512], f32, tag=f"gp{c % 2}")
            nc.tensor.matmul(pre_ps[:], w1xt[:],
                             xtb[:, c * 512:(c + 1) * 512],
                             start=True, stop=True)
            nc.scalar.activation(pre_f[:, c * 512:(c + 1) * 512], pre_ps[:],
                                 AF.Copy)

        # ---- recurrence ----
        AL = mybir.AluOpType
        pre_r = pre[:]  # (k, n, b)

        def emit_u_c(hf):
            # u_T = 0.5*W1s @ c~ (+ h-part later); scales folded host-side
            d = HV[hf]
            d["u_ps"] = ps_u.tile([TM1, 128], f32, tag=f"ups{hf}",
                                  name=f"ups{hf}")
            nc.tensor.matmul(d["u_ps"][:], w1st[:], d["cb"][:], start=True,
                             stop=False)

        def emit_u_h(hf):
            # h-part in bf16 (fp32 matmul = double LOW/HIGH pass, slower)
            d = HV[hf]
            nc.vector.tensor_copy(d["hb"][:], d["h"][:])
            nc.tensor.matmul(d["u_ps"][:], w1ht[:], d["hb"][:], start=False,
                             stop=True)
            nc.vector.tensor_scalar_add(d["u"][:], d["u_ps"][:],
                                        b1sb[:])  # downcast, b1 folded in

        def emit_adds(hf):
            # z_pre(g) = pre(g) + u (broadcast over n); tanh'd in emit_tanh_e
            d = HV[hf]
            b0 = hf * 128
            d["zt"] = []
            for g in range(NG):
                gs, go = GSIZES[g], GOFF[g]
                usrc = d["u_ps"] if g == 0 else d["u"]
                u_bc = bass.AP(tensor=usrc.tensor, offset=usrc.offset,
                               ap=[usrc.ap[0], [0, gs], [1, 128]])
                z = zpre_pool.tile([TM1, NGMAX, 128], zdt, tag=f"zpre{hf}",
                                   name=f"z{hf}_{g}")
                if g in CUSTOM_GROUPS:
                    nc.vector._custom_dve(
                        OP, out=z[:, 0:gs, :],
                        in0=pre_r[:, go:go + gs, b0:b0 + 128],
                        in1=u_bc, s0=POLY_L, s1=POLY_Q)
                else:
                    nc.vector.tensor_add(
                        z[:, 0:gs, :],
                        pre_r[:, go:go + gs, b0:b0 + 128], u_bc)
                d["zt"].append(z)

        def emit_tanh_e_group(hf, g):
            d = HV[hf]
            if g == 0:
                d["e_ps"] = ps_e.tile([128, N], f32, tag=f"emm{hf}",
                                      name=f"eps{hf}")
            e_ps = d["e_ps"]
            z = d["zt"][g]
            gs, go = GSIZES[g], GOFF[g]
            if g not in CUSTOM_GROUPS:
                # group 0 adds u straight from PSUM (no b1 yet): ACT adds it;
                # groups 1-3 read u with b1 already folded in
                bias = b1sb[:] if g == 0 else 0.0
                nc.scalar.activation(z[:, 0:gs, :], z[:, 0:gs, :], AF.Tanh,
                                     bias=bias)
            w2g = w2csb if g in CUSTOM_GROUPS else w2sb
            for j in range(gs):
                nn_ = go + j
                nc.tensor.matmul(e_ps[:, nn_:nn_ + 1], z[:, j, :],
                                 w2g[:], start=True, stop=True)

        def part_exp_attn(t, hf):
            d = HV[hf]
            # softmax over n (free dim), fold b_attn2 via ACT bias
            nc.scalar.activation(d["exp_e"][:], d["e_ps"][:], AF.Exp,
                                 bias=b2sb[:], scale=1.0,
                                 accum_out=d["s"][:])
            nc.vector.reciprocal(d["rs"][:], d["s"][:])
            # alpha = exp_e / s ; transpose ; xaug[0:64] = alpha_T * x_T
            nc.vector.tensor_scalar_mul(d["xtil"][:], d["exp_e"][:],
                                        d["rs"][:])
            xt_ps = ps_xt.tile([128, 128], zdt, tag=f"tp{hf}",
                               name=f"xtps{hf}")
            nc.tensor.transpose(xt_ps[0:N, :], d["xtil"][:], identb[:])
            nc.vector.tensor_mul(d["xaug"][0:N, :], xt_ps[0:N, :],
                                 d["xtn"][:])
            # gates psum (H, 4*128) = [i | f | g | o], weights prescaled
            gp = ps_g.tile([H, 4 * 128], f32, tag=f"gp{hf}", name=f"gp{hf}")
            d["gp"] = gp
            for gi in range(4):
                nc.tensor.matmul(gp[:, gi * 128:(gi + 1) * 128],
                                 wiht[:, gi * H:(gi + 1) * H], d["xaug"][:],
                                 start=True, stop=False)
                nc.tensor.matmul(gp[:, gi * 128:(gi + 1) * 128],
                                 whht[:, gi * H:(gi + 1) * H], d["hb"][:],
                                 start=False, stop=True)

        def part_th(hf):
            # th = [tanh(i/2) tanh(f/2) tanh(g) tanh(o/2)] (scales folded)
            d = HV[hf]
            th = d["th"]
            nc.scalar.activation(th[:], d["gp"][:], AF.Tanh)
            thi, thf_, thg = th[:, 0:128], th[:, 128:256], th[:, 256:384]
            # c~ = 0.5*(1+thf)*c~ + (1+thi)*thg
            nc.vector.scalar_tensor_tensor(d["A"][:], thf_, 1.0, d["c"][:],
                                           op0=AL.add, op1=AL.mult)
            nc.vector.scalar_tensor_tensor(d["B"][:], thi, 1.0, thg,
                                           op0=AL.add, op1=AL.mult)
            nc.vector.scalar_tensor_tensor(d["c"][:], d["A"][:], 0.5,
                                           d["B"][:], op0=AL.mult,
                                           op1=AL.add)
            nc.vector.tensor_copy(d["cb"][:], d["c"][:])
            if d["more"]:
                emit_u_c(hf)

        def part_thc(t, hf, last):
            d = HV[hf]
            b0 = hf * 128
            tho = d["th"][:, 384:512]
            nc.scalar.activation(d["thc"][:], d["c"][:], AF.Tanh, scale=0.5)
            # h~ = (1+tho)*tanh(c)
            nc.vector.scalar_tensor_tensor(d["h"][:], tho, 1.0, d["thc"][:],
                                           op0=AL.add, op1=AL.mult)
            if not last:
                emit_u_h(hf)
                emit_adds(hf)
                emit_xtn(t + 1, hf)
            else:
                nc.vector.tensor_copy(d["hb"][:], d["h"][:])

        def emit_out(t, hf):
            # write h_t = 0.5*h~ out (transpose back to (b, H)); off the
            # critical chain, normal priority
            d = HV[hf]
            b0 = hf * 128
            h_ps = ps_h.tile([128, 128], f32, tag=f"tp{hf}",
                             name=f"hps{hf}")
            nc.tensor.transpose(h_ps[:], d["h"][:], ident[:])
            hb = shout.tile([128, 128], f32, tag=f"hb{hf}", name=f"hb{hf}")
            nc.vector.tensor_scalar_mul(hb[:], h_ps[:], 0.5)
            nc.sync.dma_start(out_ap[b0:b0 + 128, t, :], hb[:])

        def emit_xtn(t, hf):
            d = HV[hf]
            b0 = hf * 128
            d["xtn"] = shout.tile([N, 128], f32, tag=f"xtn{hf}",
                                  name=f"xtn{hf}")
            nc.sync.dma_start(d["xtn"][:], xtn_ap[:, t, b0:b0 + 128])

        # Anti-phased emission: the two half-batches run half a step out of
        # phase; each half's tail ACT ops (th/thc) are spliced between the
        # other half's tanh groups so ACT never idles on the tail chains.
        for hf in range(CHUNKS):
            HV[hf]["more"] = True
            emit_u_c(hf)
            emit_u_h(hf)
            emit_adds(hf)
            emit_xtn(0, hf)
        pend = {0: None, 1: None}  # half -> step index with deferred tail
        for t in range(steps):
            for X in range(CHUNKS):
                Y = 1 - X
                emit_tanh_e_group(X, 0)
                if pend[Y] is not None:
                    HV[Y]["more"] = pend[Y] + 1 < steps
                    with tc.high_priority():
                        part_th(Y)
                emit_tanh_e_group(X, 1)
                if pend[Y] is not None:
                    with tc.high_priority():
                        part_thc(pend[Y], Y, last=(pend[Y] == steps - 1))
                    emit_out(pend[Y], Y)
                    pend[Y] = None
                for g in range(2, NG):
                    emit_tanh_e_group(X, g)
                with tc.high_priority():
                    part_exp_attn(t, X)
                pend[X] = t
        # drain the last pending tails
        for X in range(CHUNKS):
            if pend[X] is not None:
                HV[X]["more"] = False
                part_th(X)
                part_thc(pend[X], X, last=True)
                emit_out(pend[X], X)
                pend[X] = None

    nc.compile()
    return nc


def _pack_inputs(X, W_attn1, b_attn1, w_attn2, b_attn2, W_ih, W_hh, b_ih,
                 b_hh):
    """Host-side marshalling: shard X, pre-transpose the small weights."""
    f = np.float32
    W_attn1 = np.asarray(W_attn1, f)
    # State is kept as h~=2h, c~=2c and sigmoids are computed via
    # tanh(x/2): fold the needed 0.5 factors into the weights here.
    # gate scale: i,f,o rows get 0.5 (tanh(x/2) trick); g rows keep 1.
    gs = np.concatenate([np.full(H, 0.5, f), np.full(H, 0.5, f),
                         np.ones(H, f), np.full(H, 0.5, f)])  # (4H,)
    w1ht = np.ascontiguousarray(W_attn1[:, :H].T) * 0.5      # h~ = 2h
    w1st = np.ascontiguousarray(W_attn1[:, H:2 * H].T) * 0.5  # c~ = 2c
    w1xt = np.ascontiguousarray(W_attn1[:, 2 * H:].T)
    b1 = np.asarray(b_attn1, f).reshape(TM1, 1)
    w2 = np.asarray(w_attn2, f).reshape(TM1, 1)
    w2c = w2 * np.float32(POLY_C)
    b2 = np.full((128, 1), np.asarray(b_attn2, f).reshape(-1)[0], f)
    b_lstm = ((np.asarray(b_ih, f) + np.asarray(b_hh, f)) * gs).reshape(
        1, 4 * H)
    wihta = np.concatenate(
        [np.ascontiguousarray(np.asarray(W_ih, f).T) * gs, b_lstm], axis=0)
    whht = np.ascontiguousarray(np.asarray(W_hh, f).T) * gs * 0.5  # h~ = 2h
    X = np.asarray(X, f)
    maps = []
    for i in range(NCORES):
        Xc = X[i * BL:(i + 1) * BL]                        # (BL, T, N)
        xt = np.ascontiguousarray(Xc.transpose(1, 2, 0)).reshape(TM1, N * BL)
        xtn = np.ascontiguousarray(Xc.transpose(2, 1, 0))  # (N, T, BL)
        maps.append({
            "XT": xt, "XTN": xtn,
            "W1HT": w1ht, "W1ST": w1st, "W1XT": w1xt,
            "B1": b1, "W2": w2, "W2C": w2c, "B2": b2,
            "WIHTA": wihta, "WHHT": whht,
        })
    return maps


def _get_nc():
    if "nc" not in _cache:
        steps = int(os.environ.get("KERNEL_STEPS", TM1))
        _cache["nc"] = _build(steps)
    return _cache["nc"]


def run(trace=False, **inputs):
    from concourse.bass_utils import run_bass_kernel_spmd
    nc = _get_nc()
    in_maps = _pack_inputs(**inputs)
    res = run_bass_kernel_spmd(nc, in_maps, core_ids=list(range(NCORES)),
                               trace=trace)
    out = np.concatenate([res.results[i]["OUT"] for i in range(NCORES)],
                         axis=0)
    return out, res


def kernel(**inputs) -> np.ndarray:
    out, _ = run(trace=False, **inputs)
    return out
